# revision 43
# baseline (speedup 1.0000x reference)
"""Distributed Trainium2 kernel for nn_Attention_21990232555717.

Reference (per batch element a, seq b=1024, model dim c=1024, 16 heads):
    qkv = x @ w_qkv                       # (b, 3072)
    q,k,v split per head (hd=64)
    scores = q @ k.T * (1/sqrt(1024))     # (h, b, b)
    attn = softmax(scores, axis=HEADS)    # normalize across the 16 heads!
    out = attn @ v -> (b, 1024) @ w_out + b_out

Sharding: pure data parallel - batch (8) across 8 cores, weights replicated.
No collectives needed.

Per-core dataflow (f32r matmuls for projections, bf16 for the softmax path,
f32 accumulation in PSUM everywhere):
  xT   (c, s) f32r  from PE transposes of x
  QKT  (f, s) bf16  = w_qk^T @ x^T  (lhsT=w_qk f32r, rhs=xT f32r)
  Vb   (s, f) bf16  = x @ w_v       (lhsT=xT, rhs=w_v)
  scoresT (k, q) psum f32 per head  (lhsT=KT_h bf16, rhs=QT_h bf16)
  E = exp(scores/32) bf16; denom = sum_h E; attn = E * recip(denom)  [in-place]
  outT (f=h*64+d, q) = accum_k (lhsT=Vb_h bf16, rhs=attn_h bf16)
  y (s, e) = (lhsT=outT bf16, rhs=wout bf16) + b broadcast (DVE add)

Pipeline design (vs. the naive version): the PE never drains —
  per q-block: D1(scores+exp, k-group 0) | out-proj of previous q-block |
  D1(k-group 1) | attn@V first k-half (partials spilled to SBUF) |
  attn@V second k-half (merged at the outT copy).
Softmax denominator is a fused pairwise add-chain emitted incrementally as
the exps land, split across DVE and GpSimd; normalize-muls are wave-ordered
and split across DVE/GpSimd so attn@V can chase them.
"""

import numpy as np

import concourse.bass as bass
import concourse.mybir as mybir
import concourse.tile as tile
from concourse import bacc
from concourse.bass import broadcast_tensor_aps
from concourse.bass_utils import run_bass_kernel_spmd
from concourse.masks import make_identity

F32 = mybir.dt.float32
F32R = mybir.dt.float32r
BF16 = mybir.dt.bfloat16
Exp = mybir.ActivationFunctionType.Exp
Bypass = mybir.AluOpType.bypass
Add = mybir.AluOpType.add

S = 1024      # sequence length per core (batch element)
C = 1024      # model dim
H = 16        # heads
HD = 64       # head dim
SCALE = 1.0 / (C ** 0.5)
QB = 256      # q block size
NQB = S // QB          # 4 q blocks
NKT = S // 128         # 8 k tiles
NCT = C // 128         # 8 contraction tiles


def build():
    nc = bacc.Bacc(None, target_bir_lowering=False)
    x_ext = nc.declare_dram_parameter("x", [S, C], F32, isOutput=False)
    wqkv_ext = nc.declare_dram_parameter("w_qkv", [C, 3 * C], F32, isOutput=False)
    wout_ext = nc.declare_dram_parameter("w_out", [C, C], F32, isOutput=False)
    b_ext = nc.declare_dram_parameter("b_out", [C], F32, isOutput=False)
    out_ext = nc.declare_dram_parameter("out", [S, C], F32, isOutput=True)

    wqkv_r = wqkv_ext[:].bitcast(F32R)

    with tile.TileContext(nc) as tc:
        with (
            tc.tile_pool(name="const_p", bufs=1) as const_p,
            tc.tile_pool(name="act_p", bufs=1) as act_p,
        ):
            # ---- constants ----
            ident = const_p.tile([128, 128], F32)
            make_identity(nc, ident)
            ones1 = const_p.tile([1, 128], BF16)
            nc.vector.memset(ones1, 1.0)
            b_f = const_p.tile([1, C], F32)
            nc.sync.dma_start(b_f, b_ext[None, :])
            b_sb = const_p.tile([1, C], BF16)
            nc.vector.tensor_copy(b_sb, b_f)

            # ---- persistent activations ----
            QKT = act_p.tile([128, H, S], BF16)        # 4 MB  (Q tiles 0..7, K tiles 8..15)
            Vb = act_p.tile([128, NKT, C], BF16)       # 2 MB

            # ============ stages A-C: transpose x, qkv projections ============
            with (
                tc.tile_pool(name="ps_t", bufs=2, space="PSUM") as ps_t,
                tc.tile_pool(name="ps_b", bufs=2, space="PSUM") as ps_b,
                tc.tile_pool(name="xt_p", bufs=1) as xt_p,
                tc.tile_pool(name="xs_p", bufs=4) as xs_p,
                tc.tile_pool(name="w_p", bufs=1) as w_p,
            ):
                xT = xt_p.tile([128, NCT, S], F32R)        # 4 MB
                wqk = w_p.tile([128, NCT, 2 * C], F32R)    # 8 MB
                wv = w_p.tile([128, NCT, C], F32R)         # 4 MB

                # DMA issue order IS completion order (one hw queue):
                # x slabs 0-3 first (transposes start immediately), then wv
                # (v_proj), then wqk (qk_proj). Slabs 4-7 are issued from the
                # scalar engine: its stream stalls on buffer reuse without
                # holding up the wv/wqk issues on sync, and its ring runs
                # concurrently with queue 1.
                xs_tiles = []
                for st in range(NKT):
                    xs = xs_p.tile([128, C], F32, tag="xslab", name=f"xs{st}")
                    xs_tiles.append(xs)
                for st in range(4):
                    nc.sync.dma_start(xs_tiles[st], x_ext[st * 128:(st + 1) * 128, :])
                for ct in range(NCT):
                    nc.sync.dma_start(
                        wv[:, ct, :], wqkv_r[ct * 128:(ct + 1) * 128, 2 * C:3 * C])
                for ct in range(NCT):
                    nc.sync.dma_start(
                        wqk[:, ct, :], wqkv_r[ct * 128:(ct + 1) * 128, 0:2 * C])
                for st in range(4, NKT):
                    nc.scalar.dma_start(xs_tiles[st], x_ext[st * 128:(st + 1) * 128, :])

                with nc.named_scope("transpose_x"):
                    for st in range(NKT):
                        xs = xs_tiles[st]
                        for cq in range(2):
                            pt = ps_t.tile([128, 512], F32, tag="pt",
                                           name=f"pt{st}_{cq}")
                            for k in range(4):
                                ct = cq * 4 + k
                                nc.tensor.transpose(
                                    pt[:, k * 128:(k + 1) * 128],
                                    xs[:, ct * 128:(ct + 1) * 128], ident)
                            dst = xT[:, cq * 4:(cq + 1) * 4, st * 128:(st + 1) * 128]
                            ptv = pt.rearrange("p (a b) -> p a b", a=4)
                            if cq % 2 == 0:
                                nc.vector.tensor_copy(dst, ptv)
                            else:
                                nc.scalar.copy(dst, ptv)

                # ---- stage B: Vb = x @ w_v ----
                with nc.named_scope("v_proj"):
                    for st in range(NKT):
                        pss = [ps_b.tile([128, 512], F32, tag=f"psb{fb}", name=f"psc{st}_{fb}")
                               for fb in range(2)]
                        for ct in range(NCT):
                            lhsT = xT[:, ct, st * 128:(st + 1) * 128]
                            for fb in range(2):
                                nc.tensor.matmul(
                                    pss[fb], lhsT, wv[:, ct, fb * 512:(fb + 1) * 512],
                                    start=(ct == 0), stop=(ct == NCT - 1),
                                )
                        for fb in range(2):
                            if st % 2 == 0:
                                nc.scalar.copy(Vb[:, st, fb * 512:(fb + 1) * 512], pss[fb])
                            else:
                                nc.vector.tensor_copy(Vb[:, st, fb * 512:(fb + 1) * 512], pss[fb])

                # ---- stage C: QKT = w_qk^T @ x^T ----
                with nc.named_scope("qk_proj"):
                    for ft in range(H):
                        pss = [ps_b.tile([128, 512], F32, tag=f"psb{sb}", name=f"psb{ft}_{sb}")
                               for sb in range(2)]
                        for ct in range(NCT):
                            lhsT = wqk[:, ct, ft * 128:(ft + 1) * 128]
                            for sb in range(2):
                                nc.tensor.matmul(
                                    pss[sb], lhsT, xT[:, ct, sb * 512:(sb + 1) * 512],
                                    start=(ct == 0), stop=(ct == NCT - 1),
                                )
                        for sb in range(2):
                            if ft % 2 == 0:
                                nc.scalar.copy(QKT[:, ft, sb * 512:(sb + 1) * 512], pss[sb])
                            else:
                                nc.vector.tensor_copy(QKT[:, ft, sb * 512:(sb + 1) * 512], pss[sb])

            # ================= stage D/E: attention + out proj =================
            with (
                tc.tile_pool(name="ps_s", bufs=2, space="PSUM") as ps_s,
                tc.tile_pool(name="ps_a", bufs=4, space="PSUM") as ps_a,
                tc.tile_pool(name="wout_p", bufs=1) as wout_p,
                tc.tile_pool(name="e_pool", bufs=1) as e_pool,
                tc.tile_pool(name="d_pool", bufs=1) as d_pool,
                tc.tile_pool(name="o_pool", bufs=2) as o_pool,
                tc.tile_pool(name="y_pool", bufs=2) as y_pool,
            ):
                wout = wout_p.tile([128, NCT, C], BF16)        # 2 MB
                partials = wout_p.tile([128, NKT, QB], F32)    # 1 MB attn@V k-half
                b_bcast = wout_p.tile([128, C], F32)           # bias on all partitions
                for ec in range(2):
                    psb = ps_a.tile([128, 512], F32, tag="acc", name=f"psbb{ec}")
                    nc.tensor.matmul(psb, ones1, b_sb[:, ec * 512:(ec + 1) * 512],
                                     start=True, stop=True)
                    nc.vector.tensor_copy(b_bcast[:, ec * 512:(ec + 1) * 512], psb)

                # wout load: DMA f32 staging + casts; casts are emitted inside
                # qb0 (below) so they don't block the denominator ops.
                wt_tiles = []
                for ftc in range(NCT):
                    wt = y_pool.tile([128, C], F32, tag="wtmp", name=f"wt{ftc}", bufs=2)
                    nc.sync.dma_start(wt, wout_ext[ftc * 128:(ftc + 1) * 128, :])
                    wt_tiles.append((ftc, wt))

                def emit_out_proj_unit(q0, outT, qsub, ec):
                    psy = ps_a.tile([128, 512], F32, tag="acc",
                                    name=f"psy{q0}_{qsub}_{ec}")
                    for ft in range(NCT):
                        nc.tensor.matmul(
                            psy,
                            outT[:, ft, qsub * 128:(qsub + 1) * 128],
                            wout[:, ft, ec * 512:(ec + 1) * 512],
                            start=(ft == 0), stop=(ft == NCT - 1),
                        )
                    y = y_pool.tile([128, 512], F32, tag="y",
                                    name=f"y{q0}_{qsub}_{ec}")
                    nc.vector.scalar_tensor_tensor(
                        y, psy, 0.0, b_bcast[:, ec * 512:(ec + 1) * 512],
                        Bypass, Add)
                    nc.sync.dma_start(
                        out_ext[q0 + qsub * 128:q0 + (qsub + 1) * 128,
                                ec * 512:(ec + 1) * 512],
                        y,
                    )

                def emit_out_proj(q0, outT):
                    with nc.named_scope(f"out_proj_q{q0}"):
                        for qsub in range(QB // 128):
                            for ec in range(2):
                                emit_out_proj_unit(q0, outT, qsub, ec)

                pending = None      # deferred out-proj: (q0, outT)
                pending_d3b = None  # deferred attn@V half B: (outT, Etiles, qb)
                for qb in range(NQB):
                    q0 = qb * QB
                    Etiles = {}

                    def emit_scores(h, gg, qb=qb, q0=q0, Etiles=Etiles):
                        po = 64 * (h % 2)
                        rhs = QKT[po:po + 64, h // 2, q0:q0 + QB]
                        pss = ps_s.tile([128, 4 * QB], F32, tag="scores",
                                        name=f"sc{qb}_{gg}_{h}")
                        for j in range(4):
                            kt = 4 * gg + j
                            lhsT = QKT[po:po + 64, 8 + h // 2, kt * 128:(kt + 1) * 128]
                            nc.tensor.matmul(pss[:, j * QB:(j + 1) * QB], lhsT, rhs,
                                             start=True, stop=True)
                        et = Etiles[(h, gg)]
                        nc.scalar.activation(et, pss, Exp, scale=SCALE)

                    def emit_den_link(h, gg, E, dl):
                        """Incremental denominator chain after exp of head h.
                        dl[j] accumulates heads 2j,2j+1,2j+8,2j+9.  g0's dl[0]
                        chain goes to GpSimd (off-critical); g1's chains all
                        stay on DVE - its tail gates D3b(prev) and the qb
                        boundary, and GpSimd's ~2.8us ops would stretch it."""
                        i = h // 2
                        j = i if h < 8 else i - 4
                        eng = nc.vector
                        if h % 2 == 1 and h < 8:
                            eng.tensor_add(dl[:, j, :], E[:, 2 * j, :], E[:, 2 * j + 1, :])
                        elif h >= 8:
                            eng.tensor_add(dl[:, j, :], dl[:, j, :], E[:, h, :])

                    def emit_d3_wave(w, half, outT, Etiles_=None, qb_=None,
                                     qb=qb, Etiles=Etiles):
                        if Etiles_ is None:
                            Etiles_, qb_ = Etiles, qb
                        aw = ps_a.tile([128, 512], F32, tag="acc",
                                       name=f"acc{qb_}_{half}_{w}")
                        # each packed head opens its own accumulation group on
                        # its disjoint partition half (start=True zeroes only
                        # the written region) - no memset needed
                        kts = range(4 * half, 4 * half + 4)
                        for kt in kts:
                            gg, j = kt // 4, kt % 4
                            for i in range(2):
                                h = 2 * w + i
                                po = 64 * (h % 2)
                                nc.tensor.matmul(
                                    aw[po:po + 64, 0:QB],
                                    Vb[:, kt, h * HD:(h + 1) * HD],
                                    Etiles_[(h, gg)][:, j * QB:(j + 1) * QB],
                                    start=(kt == kts[0]),
                                    stop=(kt == kts[-1]),
                                    tile_position=(0, po),
                                )
                        if half == 0:
                            nc.scalar.copy(partials[:, w, :], aw[:, 0:QB])
                        else:
                            # outT[:, w, :] = partials + psum (cast to bf16)
                            nc.vector.scalar_tensor_tensor(
                                outT[:, w, :], partials[:, w, :], 0.0,
                                aw[:, 0:QB], Bypass, Add)

                    with nc.named_scope(f"attn_qb{qb}"):
                        outT = o_pool.tile([128, NCT, QB], BF16, tag="outT",
                                           name=f"outT{qb}")
                        Eg = {}
                        dlg = {}
                        recg = {}
                        for gg in range(2):
                            Eg[gg] = e_pool.tile([128, H, 4 * QB], BF16,
                                                 tag=f"E{gg}", name=f"E{qb}_{gg}")
                            dlg[gg] = d_pool.tile([128, 4, 4 * QB], BF16,
                                                  tag=f"dl{gg}",
                                                  name=f"dl{qb}_{gg}", bufs=1)
                            for h in range(H):
                                Etiles[(h, gg)] = Eg[gg][:, h, :]

                        def den_tail_and_muls(gg):
                            dl = dlg[gg]
                            E = Eg[gg]
                            nc.vector.tensor_add(dl[:, 2, :], dl[:, 2, :], dl[:, 3, :])
                            denf = d_pool.tile([128, 4 * QB], F32, tag="denf",
                                               name=f"denf{qb}_{gg}", bufs=2)
                            nc.vector.tensor_add(denf, dl[:, 0, :], dl[:, 2, :])
                            rec_f = d_pool.tile([128, 4 * QB], F32, tag="recf",
                                                bufs=2, name=f"recf{qb}_{gg}")
                            nc.vector.reciprocal_approx_fast(out=rec_f, in_=denf)
                            rec = d_pool.tile([128, 4 * QB], BF16, tag="rec",
                                              bufs=2, name=f"rec{qb}_{gg}")
                            nc.vector.tensor_copy(rec, rec_f)
                            recg[gg] = rec
                            # wave-ordered normalize: pair, pair, quad, quad, quad
                            groups = [(0, 2), (2, 4), (4, 8), (8, 12), (12, 16)]
                            for h0, h1 in groups:
                                esl = E[:, h0:h1, :]
                                rb, _ = broadcast_tensor_aps(rec[:, None, :], esl)
                                nc.vector.tensor_mul(esl, esl, rb)

                        # ---- D1 g0 with D3b(prev) waves interleaved at a
                        # 4-pair lag (D3b reads only E_g1[prev], so it must
                        # finish before THIS qb's g1 exps, not g0's; the lag
                        # keeps the first scores from gating on muls_g1(prev))
                        for h in range(H):
                            emit_scores(h, 0)
                            emit_den_link(h, 0, Eg[0], dlg[0])
                            if h == 13:
                                # t01 folded as dl0+dl1 -> dl0
                                nc.vector.tensor_add(dlg[0][:, 0, :],
                                                     dlg[0][:, 0, :], dlg[0][:, 1, :])
                            if pending_d3b is not None and h % 2 == 1:
                                w = (h - 1) // 2 - 3
                                if w >= 0:
                                    po_, pe_, pq_ = pending_d3b
                                    emit_d3_wave(w, 1, po_, pe_, pq_)
                        if pending_d3b is not None:
                            po_, pe_, pq_ = pending_d3b
                            for w in range(4, NKT):
                                emit_d3_wave(w, 1, po_, pe_, pq_)
                        pending_d3b = None
                        den_tail_and_muls(0)
                        # OP(prev) unit 0 fits here: all D3b(prev) merges for
                        # outT(prev) just completed in this g0 window
                        if pending is not None:
                            emit_out_proj_unit(pending[0], pending[1], 0, 0)
                        # ---- D1 g1: D3a waves (2-pair lag) + OP(prev) units ----
                        wq = []
                        for h in range(H):
                            emit_scores(h, 1)
                            emit_den_link(h, 1, Eg[1], dlg[1])
                            if h == 13:
                                nc.vector.tensor_add(dlg[1][:, 0, :],
                                                     dlg[1][:, 0, :], dlg[1][:, 1, :])
                            if h % 2 == 1:
                                w = (h - 1) // 2 - 2
                                if w >= 0:
                                    emit_d3_wave(w, 0, outT)
                                    wq.append(w)
                            if pending is not None and h % 4 == 3 and h < 15:
                                u = h // 4 + 1
                                emit_out_proj_unit(pending[0], pending[1],
                                                   u // 2, u % 2)
                        pending = None
                        for w in range(NKT):
                            if w not in wq:
                                emit_d3_wave(w, 0, outT)
                        den_tail_and_muls(1)
                        # ---- D3b deferred into the next qb's D1 g0 ----
                        if qb == 0:
                            # wout casts: off-critical, split ACT/DVE
                            for ftc, wt in wt_tiles:
                                if ftc % 2 == 0:
                                    nc.scalar.copy(wout[:, ftc, :], wt)
                                else:
                                    nc.vector.tensor_copy(wout[:, ftc, :], wt)
                    pending = (q0, outT)
                    pending_d3b = (outT, Etiles, qb)
                # tail: the last qb's deferred D3b, then its out-proj
                po_, pe_, pq_ = pending_d3b
                for w in range(NKT):
                    emit_d3_wave(w, 1, po_, pe_, pq_)
                emit_out_proj(*pending)

    nc.compile()
    return nc


_NC = None


def _get_nc():
    global _NC
    if _NC is None:
        _NC = build()
    return _NC


def kernel(x, w_qkv, w_out, b_out):
    nc = _get_nc()
    x = np.ascontiguousarray(np.asarray(x, dtype=np.float32))
    w_qkv = np.ascontiguousarray(np.asarray(w_qkv, dtype=np.float32))
    w_out = np.ascontiguousarray(np.asarray(w_out, dtype=np.float32))
    b_out = np.ascontiguousarray(np.asarray(b_out, dtype=np.float32))
    in_maps = [
        {"x": x[i], "w_qkv": w_qkv, "w_out": w_out, "b_out": b_out}
        for i in range(8)
    ]
    res = run_bass_kernel_spmd(nc, in_maps, core_ids=list(range(8)))
    out = np.stack([np.asarray(res.results[i]["out"]) for i in range(8)])
    return out.astype(np.float32)
